# revision 8
# baseline (speedup 1.0000x reference)
"""GRU seq2seq autoencoder (B=1024, T=512, C=32, H=256) on 8 trn2 NeuronCores.

Strategy: data-parallel over batch (128 rows/core, weights replicated).
Per-core layout is feature-major: hidden state h lives in SBUF as
[128 partitions, 2*128] where column-chunk k holds features 128k..128k+127
for all 128 batch rows. All matmuls are out[features, batch] =
W_chunk @ h (lhsT = W.T chunk stationary, rhs = h chunk streaming), so the
recurrence needs no transposes anywhere.

Bias folding: input-projection biases (+ r/z recurrent biases) ride on an
augmented ones-row of x through the Wih matmul; the n-gate recurrent bias
(inside the r* product) is added via a K=1 rank-1 matmul into the same PSUM
accumulation group. Gates then need only: one sigmoid over [128,512] (r|z),
2 DVE tensor ops, one tanh over [128,256], and 3 DVE tensor ops per step.

Matmul inputs and gate tiles are fp16 (PE streams 16-bit at 4x the fp32
rate; DVE tensor_tensor gets 2x mode); PSUM accumulation stays fp32.
"""

import os

import ml_dtypes
import numpy as np

import concourse.bacc as bacc
import concourse.mybir as mybir
import concourse.tile as tile
from concourse.bass_utils import run_bass_kernel_spmd

B, T, C, H = 1024, 512, 32, 256
NCORES = 8
BC = B // NCORES  # batch per core = 128
CA = C + 1  # augmented input rows (ones row carries biases)
XBLK = 32  # timesteps per x-stream DMA block
F32 = mybir.dt.float32
AF = mybir.ActivationFunctionType
OP = mybir.AluOpType

# Best measured config (A/B on hardware): split r/z sigmoid (shorter
# dependency chain), keep all gate tensor ops on the vector engine
# (GPSIMD offload loses to SBUF-port contention).
SPLIT_SIG = True
GP_OFFLOAD = False
ENC_STREAM = True

MM_DT = mybir.dt.float16
NP_MM = ml_dtypes.float16 if hasattr(ml_dtypes, "float16") else np.float16
GATE_DT = MM_DT  # dtype of rz/n/t1/q/d/e/h tiles


def build(t_steps=T, reps=1):
    nblk = (t_steps + XBLK - 1) // XBLK
    assert t_steps % XBLK == 0 or t_steps < XBLK
    xblk = min(XBLK, t_steps)
    nc = bacc.Bacc("TRN2", num_devices=NCORES)

    xd = nc.dram_tensor("x_t", [nblk, CA, xblk * BC], MM_DT, kind="ExternalInput").ap()
    whh_e_d = nc.dram_tensor("whh_e", [128, 12 * 128], MM_DT, kind="ExternalInput").ap()
    whh_d_d = nc.dram_tensor("whh_d", [128, 12 * 128], MM_DT, kind="ExternalInput").ap()
    wih_e_d = nc.dram_tensor("wih_e", [CA, 768], MM_DT, kind="ExternalInput").ap()
    wih_d_d = nc.dram_tensor("wih_d", [CA, 768], MM_DT, kind="ExternalInput").ap()
    bhn_e_d = nc.dram_tensor("bhn_e", [1, 256], MM_DT, kind="ExternalInput").ap()
    bhn_d_d = nc.dram_tensor("bhn_d", [1, 256], MM_DT, kind="ExternalInput").ap()
    projT_d = nc.dram_tensor("projT", [128, 64], MM_DT, kind="ExternalInput").ap()
    projb_d = nc.dram_tensor("projb", [32, 1], F32, kind="ExternalInput").ap()
    yd = nc.dram_tensor("y_t", [t_steps, C, BC], F32, kind="ExternalOutput").ap()

    with tile.TileContext(nc) as tc:
        with (
            tc.tile_pool(name="const", bufs=1) as constp,
            tc.tile_pool(name="xp", bufs=2) as xp,
            tc.tile_pool(name="state", bufs=2) as statep,
            tc.tile_pool(name="work", bufs=2) as workp,
            tc.tile_pool(name="psum", bufs=2, space="PSUM") as psump,
        ):
            whh_e = constp.tile([128, 1536], MM_DT)
            nc.sync.dma_start(whh_e[:], whh_e_d[:])
            whh_d = constp.tile([128, 1536], MM_DT)
            nc.sync.dma_start(whh_d[:], whh_d_d[:])
            wih_e = constp.tile([CA, 768], MM_DT)
            nc.sync.dma_start(wih_e[:], wih_e_d[:])
            wih_d = constp.tile([CA, 768], MM_DT)
            nc.sync.dma_start(wih_d[:], wih_d_d[:])
            bhn_e = constp.tile([1, 256], MM_DT)
            nc.sync.dma_start(bhn_e[:], bhn_e_d[:])
            bhn_d = constp.tile([1, 256], MM_DT)
            nc.sync.dma_start(bhn_d[:], bhn_d_d[:])
            projT = constp.tile([128, 64], MM_DT)
            nc.sync.dma_start(projT[:], projT_d[:])
            projb = constp.tile([32, 1], F32)
            nc.sync.dma_start(projb[:], projb_d[:])
            ones_row = constp.tile([1, BC], MM_DT)
            nc.vector.memset(ones_row[:], 1.0)
            dec_in = constp.tile([CA, BC], MM_DT)
            nc.vector.memset(dec_in[C : C + 1, :], 1.0)

            def gru_step(wh, wi, bhn, x_ap, h_prev, gi_first):
                # PSUM accumulation groups must be sequential per bank (2KB
                # "zero region"): each region's [open ... close] matmuls stay
                # contiguous in PE program order.
                psum_rz = psump.tile([128, 512], F32, name="psum_rz")
                psum_n = psump.tile([128, 512], F32, name="psum_n")

                def rz_groups(ms):
                    for m in ms:
                        seg = psum_rz[:, m * 128 : (m + 1) * 128]
                        gi = (
                            wi[:, m * 128 : (m + 1) * 128], x_ap,
                        )
                        wh0 = (
                            wh[:, (m * 2) * 128 : (m * 2 + 1) * 128],
                            h_prev[:, 0:128],
                        )
                        wh1 = (
                            wh[:, (m * 2 + 1) * 128 : (m * 2 + 2) * 128],
                            h_prev[:, 128:256],
                        )
                        ops = [gi, wh0, wh1] if gi_first else [wh0, wh1, gi]
                        for i, (lhsT, rhs) in enumerate(ops):
                            nc.tensor.matmul(
                                seg, lhsT, rhs, start=(i == 0), stop=(i == 2)
                            )

                def ghn_groups():
                    for cc in range(2):
                        seg = psum_n[:, cc * 128 : (cc + 1) * 128]
                        m = 4 + cc
                        nc.tensor.matmul(
                            seg, bhn[:, cc * 128 : (cc + 1) * 128], ones_row[:],
                            start=True, stop=False,
                        )
                        nc.tensor.matmul(
                            seg, wh[:, (m * 2) * 128 : (m * 2 + 1) * 128],
                            h_prev[:, 0:128], start=False, stop=False,
                        )
                        nc.tensor.matmul(
                            seg, wh[:, (m * 2 + 1) * 128 : (m * 2 + 2) * 128],
                            h_prev[:, 128:256], start=False, stop=True,
                        )

                def gin_groups():
                    for cc in range(2):
                        nc.tensor.matmul(
                            psum_n[:, 256 + cc * 128 : 256 + (cc + 1) * 128],
                            wi[:, (4 + cc) * 128 : (5 + cc) * 128], x_ap,
                            start=True, stop=True,
                        )

                # PE order: r regions first (unblocks sig_r), then ghn (t1's
                # other input), then z regions, then gin. Decoder puts ghn
                # first so pred-independent work hides the pred->gi latency.
                if gi_first:
                    rz_groups([0, 1]); ghn_groups(); rz_groups([2, 3]); gin_groups()
                else:
                    ghn_groups(); rz_groups([0, 1]); rz_groups([2, 3]); gin_groups()

                rz = workp.tile([128, 512], GATE_DT, name="rz")
                r_ap, z_ap = rz[:, 0:256], rz[:, 256:512]
                t1 = workp.tile([128, 256], GATE_DT, name="t1")
                if SPLIT_SIG:
                    nc.scalar.activation(r_ap, psum_rz[:, 0:256], AF.Sigmoid)
                    nc.vector.tensor_tensor(t1[:], psum_n[:, 0:256], r_ap, OP.mult)
                    nc.scalar.activation(z_ap, psum_rz[:, 256:512], AF.Sigmoid)
                else:
                    nc.scalar.activation(rz[:], psum_rz[:], AF.Sigmoid)
                    nc.vector.tensor_tensor(t1[:], psum_n[:, 0:256], r_ap, OP.mult)
                q = workp.tile([128, 256], GATE_DT, name="q")
                nc.vector.tensor_tensor(q[:], t1[:], psum_n[:, 256:512], OP.add)
                # zb = 1 - z and c1 = z*h run during the tanh window
                eng = nc.gpsimd if GP_OFFLOAD else nc.vector
                zb = workp.tile([128, 256], GATE_DT, name="zb")
                eng.tensor_scalar(zb[:], z_ap, -1.0, 1.0, OP.mult, OP.add)
                c1 = workp.tile([128, 256], GATE_DT, name="c1")
                eng.tensor_tensor(c1[:], z_ap, h_prev[:], OP.mult)
                n_t = workp.tile([128, 256], GATE_DT, name="n_t")
                nc.scalar.activation(n_t[:], q[:], AF.Tanh)
                u_t = workp.tile([128, 256], GATE_DT, name="u_t")
                nc.vector.tensor_tensor(u_t[:], zb[:], n_t[:], OP.mult)
                h_new = statep.tile([128, 256], GATE_DT, name="h")
                nc.vector.tensor_add(h_new[:], c1[:], u_t[:])
                return h_new

            def body():
                nc.vector.memset(dec_in[0:C, :], 0.0)
                h = statep.tile([128, 256], GATE_DT, name="h")
                nc.vector.memset(h[:], 0.0)

                # ---- encoder ----
                for blk in range(nblk):
                    xb = xp.tile([CA, xblk * BC], MM_DT, name="xb")
                    nc.sync.dma_start(xb[:], xd[blk])
                    for j in range(xblk):
                        if blk * xblk + j >= t_steps:
                            break
                        h = gru_step(
                            whh_e, wih_e, bhn_e, xb[:, j * BC : (j + 1) * BC], h, True
                        )

                # ---- decoder ----
                for t in range(t_steps):
                    h = gru_step(whh_d, wih_d, bhn_d, dec_in[:], h, False)
                    psum_p = psump.tile([32, BC], F32, name="psum_p", bufs=2)
                    nc.tensor.matmul(
                        psum_p[:], projT[:, 0:32], h[:, 0:128], start=True, stop=False
                    )
                    nc.tensor.matmul(
                        psum_p[:], projT[:, 32:64], h[:, 128:256],
                        start=False, stop=True,
                    )
                    # on-chain: feed pred straight into dec_in (fp16);
                    # off-chain: fp32 copy for the y output DMA
                    nc.vector.tensor_scalar_add(dec_in[0:C, :], psum_p[:], projb[:])
                    pred = workp.tile([32, BC], F32, name="pred")
                    nc.vector.tensor_scalar_add(pred[:], psum_p[:], projb[:])
                    nc.sync.dma_start(yd[t], pred[:])

            if reps == 1:
                body()
            else:
                with tc.For_i(0, reps):
                    body()

    nc.compile()
    return nc




def build_2g(t_steps=T, reps=1):
    """Two-group (batch 64+64) software-pipelined variant: two independent
    recurrence chains per core fill each other's cross-engine latency."""
    G = BC // 2  # 64
    nblk = (t_steps + XBLK - 1) // XBLK
    assert t_steps % XBLK == 0 or t_steps < XBLK
    xblk = min(XBLK, t_steps)
    nc = bacc.Bacc("TRN2", num_devices=NCORES)

    xd = nc.dram_tensor("x_t", [nblk, CA, xblk * BC], MM_DT, kind="ExternalInput").ap()
    whh_e_d = nc.dram_tensor("whh_e", [128, 12 * 128], MM_DT, kind="ExternalInput").ap()
    whh_d_d = nc.dram_tensor("whh_d", [128, 12 * 128], MM_DT, kind="ExternalInput").ap()
    wih_e_d = nc.dram_tensor("wih_e", [CA, 768], MM_DT, kind="ExternalInput").ap()
    wih_d_d = nc.dram_tensor("wih_d", [CA, 768], MM_DT, kind="ExternalInput").ap()
    bhn_e_d = nc.dram_tensor("bhn_e", [1, 256], MM_DT, kind="ExternalInput").ap()
    bhn_d_d = nc.dram_tensor("bhn_d", [1, 256], MM_DT, kind="ExternalInput").ap()
    projT_d = nc.dram_tensor("projT", [128, 64], MM_DT, kind="ExternalInput").ap()
    projb_d = nc.dram_tensor("projb", [32, 1], F32, kind="ExternalInput").ap()
    yd = nc.dram_tensor("y_t", [t_steps, C, BC], F32, kind="ExternalOutput").ap()

    with tile.TileContext(nc) as tc:
        with (
            tc.tile_pool(name="const", bufs=1) as constp,
            tc.tile_pool(name="xp", bufs=2) as xp,
            tc.tile_pool(name="state", bufs=2) as statep,
            tc.tile_pool(name="work", bufs=2) as workp,
            tc.tile_pool(name="psum", bufs=2, space="PSUM") as psump,
        ):
            whh_e = constp.tile([128, 1536], MM_DT)
            nc.sync.dma_start(whh_e[:], whh_e_d[:])
            whh_d = constp.tile([128, 1536], MM_DT)
            nc.sync.dma_start(whh_d[:], whh_d_d[:])
            wih_e = constp.tile([CA, 768], MM_DT)
            nc.sync.dma_start(wih_e[:], wih_e_d[:])
            wih_d = constp.tile([CA, 768], MM_DT)
            nc.sync.dma_start(wih_d[:], wih_d_d[:])
            bhn_e = constp.tile([1, 256], MM_DT)
            nc.sync.dma_start(bhn_e[:], bhn_e_d[:])
            bhn_d = constp.tile([1, 256], MM_DT)
            nc.sync.dma_start(bhn_d[:], bhn_d_d[:])
            projT = constp.tile([128, 64], MM_DT)
            nc.sync.dma_start(projT[:], projT_d[:])
            projb = constp.tile([32, 1], F32)
            nc.sync.dma_start(projb[:], projb_d[:])
            ones_row = constp.tile([1, G], MM_DT)
            nc.vector.memset(ones_row[:], 1.0)
            dec_in = constp.tile([CA, BC], MM_DT)
            nc.vector.memset(dec_in[C : C + 1, :], 1.0)

            def emit_pe(wh, wi, bhn, x_ap, h_prev, gi_first, psum_rz, psum_n):
                # h_prev: [128, 2*G]; x_ap: [CA, G]
                def rz_group(m):
                    seg = psum_rz[:, m * G : (m + 1) * G]
                    gi = (wi[:, m * 128 : (m + 1) * 128], x_ap)
                    wh0 = (wh[:, (m * 2) * 128 : (m * 2 + 1) * 128], h_prev[:, 0:G])
                    wh1 = (
                        wh[:, (m * 2 + 1) * 128 : (m * 2 + 2) * 128],
                        h_prev[:, G : 2 * G],
                    )
                    ops = [gi, wh0, wh1] if gi_first else [wh0, wh1, gi]
                    for i, (lhsT, rhs) in enumerate(ops):
                        nc.tensor.matmul(seg, lhsT, rhs, start=(i == 0), stop=(i == 2))

                def ghn_group(cc):
                    seg = psum_n[:, cc * G : (cc + 1) * G]
                    m = 4 + cc
                    nc.tensor.matmul(
                        seg, bhn[:, cc * 128 : (cc + 1) * 128], ones_row[:],
                        start=True, stop=False,
                    )
                    nc.tensor.matmul(
                        seg, wh[:, (m * 2) * 128 : (m * 2 + 1) * 128],
                        h_prev[:, 0:G], start=False, stop=False,
                    )
                    nc.tensor.matmul(
                        seg, wh[:, (m * 2 + 1) * 128 : (m * 2 + 2) * 128],
                        h_prev[:, G : 2 * G], start=False, stop=True,
                    )

                def gin_group(cc):
                    nc.tensor.matmul(
                        psum_n[:, 2 * G + cc * G : 2 * G + (cc + 1) * G],
                        wi[:, (4 + cc) * 128 : (5 + cc) * 128], x_ap,
                        start=True, stop=True,
                    )

                if gi_first:
                    for m in (0, 1):
                        rz_group(m)
                    ghn_group(0); ghn_group(1)
                    for m in (2, 3):
                        rz_group(m)
                    gin_group(0); gin_group(1)
                else:
                    ghn_group(0); ghn_group(1)
                    for m in (0, 1, 2, 3):
                        rz_group(m)
                    gin_group(0); gin_group(1)

            def gates_front(psum_rz, psum_n, g):
                rz = workp.tile([128, 2 * 256 // 2], GATE_DT, name=f"rz{g}")
                nc.scalar.activation(rz[:], psum_rz[:], AF.Sigmoid)
                return rz

            def gates_mid(psum_n, rz, g):
                t1 = workp.tile([128, 2 * G], GATE_DT, name=f"t1{g}")
                nc.vector.tensor_tensor(
                    t1[:], psum_n[:, 0 : 2 * G], rz[:, 0 : 2 * G], OP.mult
                )
                q = workp.tile([128, 2 * G], GATE_DT, name=f"q{g}")
                nc.vector.tensor_tensor(
                    q[:], t1[:], psum_n[:, 2 * G : 4 * G], OP.add
                )
                return q

            def gates_tanh(q, g):
                n_t = workp.tile([128, 2 * G], GATE_DT, name=f"n{g}")
                nc.scalar.activation(n_t[:], q[:], AF.Tanh)
                return n_t

            def gates_tail(h_prev, n_t, rz, g):
                d_t = workp.tile([128, 2 * G], GATE_DT, name=f"d{g}")
                nc.vector.tensor_tensor(d_t[:], h_prev[:], n_t[:], OP.subtract)
                e_t = workp.tile([128, 2 * G], GATE_DT, name=f"e{g}")
                nc.vector.tensor_tensor(e_t[:], d_t[:], rz[:, 2 * G : 4 * G], OP.mult)
                h_new = statep.tile([128, 2 * G], GATE_DT, name=f"h{g}")
                nc.vector.tensor_add(h_new[:], e_t[:], n_t[:])
                return h_new

            def step_pair(wh, wi, bhn, x_aps, hs, gi_first):
                prz = [
                    psump.tile([128, 4 * G], F32, name=f"psum_rz{g}")
                    for g in range(2)
                ]
                pn = [
                    psump.tile([128, 6 * G], F32, name=f"psum_n{g}")
                    for g in range(2)
                ]
                for g in range(2):
                    emit_pe(wh, wi, bhn, x_aps[g], hs[g], gi_first, prz[g], pn[g])
                rzA = gates_front(prz[0], pn[0], 0)
                qA = gates_mid(pn[0], rzA, 0)
                rzB = gates_front(prz[1], pn[1], 1)
                nA = gates_tanh(qA, 0)
                qB = gates_mid(pn[1], rzB, 1)
                hA = gates_tail(hs[0], nA, rzA, 0)
                nB = gates_tanh(qB, 1)
                hB = gates_tail(hs[1], nB, rzB, 1)
                return [hA, hB], pn

            hs = None

            def body():
                nonlocal hs
                nc.vector.memset(dec_in[0:C, :], 0.0)
                h0 = statep.tile([128, 2 * G], GATE_DT, name="h0")
                nc.vector.memset(h0[:], 0.0)
                h1 = statep.tile([128, 2 * G], GATE_DT, name="h1")
                nc.vector.memset(h1[:], 0.0)
                hs = [h0, h1]

                for blk in range(nblk):
                    xb = xp.tile([CA, xblk * BC], MM_DT, name="xb")
                    nc.sync.dma_start(xb[:], xd[blk])
                    for j in range(xblk):
                        if blk * xblk + j >= t_steps:
                            break
                        x_aps = [
                            xb[:, j * BC + g * G : j * BC + (g + 1) * G]
                            for g in range(2)
                        ]
                        hs, _ = step_pair(whh_e, wih_e, bhn_e, x_aps, hs, True)

                for t in range(t_steps):
                    x_aps = [dec_in[:, g * G : (g + 1) * G] for g in range(2)]
                    hs, pn = step_pair(whh_d, wih_d, bhn_d, x_aps, hs, False)
                    for g in range(2):
                        pred_seg = pn[g][0:32, 4 * G : 4 * G + G]
                        nc.tensor.matmul(
                            pred_seg, projT[:, 0:32], hs[g][:, 0:G],
                            start=True, stop=False,
                        )
                        nc.tensor.matmul(
                            pred_seg, projT[:, 32:64], hs[g][:, G : 2 * G],
                            start=False, stop=True,
                        )
                        nc.vector.tensor_scalar_add(
                            dec_in[0:C, g * G : (g + 1) * G], pred_seg, projb[:]
                        )
                        pred = workp.tile([32, G], F32, name=f"pred{g}")
                        nc.vector.tensor_scalar_add(pred[:], pred_seg, projb[:])
                        nc.sync.dma_start(yd[t][:, g * G : (g + 1) * G], pred[:])

            if reps == 1:
                body()
            else:
                with tc.For_i(0, reps):
                    body()

    nc.compile()
    return nc


def build_v2(t_steps=T, reps=1):
    """v2: split PSUM tiles (r/z/ghn/gin) so each consumer waits only on its
    own producer matmuls; decoder folds Wih@proj into the recurrent weights so
    the pred->gi round-trip leaves the serial chain; pred bias-add on ACT;
    y DMA batched every YB steps."""
    YB = 8
    nblk = (t_steps + XBLK - 1) // XBLK
    assert t_steps % XBLK == 0 or t_steps < XBLK
    xblk = min(XBLK, t_steps)
    nyb = (t_steps + YB - 1) // YB
    nc = bacc.Bacc("TRN2", num_devices=NCORES)

    xd = nc.dram_tensor("x_t", [nblk, CA, xblk * BC], MM_DT, kind="ExternalInput").ap()
    whh_e_d = nc.dram_tensor("whh_e", [128, 12 * 128], MM_DT, kind="ExternalInput").ap()
    whh_d_d = nc.dram_tensor("whh_d", [128, 12 * 128], MM_DT, kind="ExternalInput").ap()
    # fused decoder weights: m-chunks 0..3 = (Whh + Wih@P)_rz, 4..5 = Whh_n
    whh_f_d = nc.dram_tensor("whh_f", [128, 12 * 128], MM_DT, kind="ExternalInput").ap()
    # (Wih@P)_n packed as 4 chunks [k-chunk*2 + m-chunk]
    wip_n_d = nc.dram_tensor("wip_n", [128, 4 * 128], MM_DT, kind="ExternalInput").ap()
    wih_e_d = nc.dram_tensor("wih_e", [CA, 768], MM_DT, kind="ExternalInput").ap()
    wih_d_d = nc.dram_tensor("wih_d", [CA, 768], MM_DT, kind="ExternalInput").ap()
    bhn_e_d = nc.dram_tensor("bhn_e", [1, 256], MM_DT, kind="ExternalInput").ap()
    bhn_d_d = nc.dram_tensor("bhn_d", [1, 256], MM_DT, kind="ExternalInput").ap()
    brz_f_d = nc.dram_tensor("brz_f", [1, 512], MM_DT, kind="ExternalInput").ap()
    bgin_f_d = nc.dram_tensor("bgin_f", [1, 256], MM_DT, kind="ExternalInput").ap()
    projT_d = nc.dram_tensor("projT", [128, 64], MM_DT, kind="ExternalInput").ap()
    projb_d = nc.dram_tensor("projb", [32, 1], F32, kind="ExternalInput").ap()
    yd = nc.dram_tensor("y_t", [nyb, C, YB * BC], F32, kind="ExternalOutput").ap()

    with tile.TileContext(nc) as tc:
        with (
            tc.tile_pool(name="const", bufs=1) as constp,
            tc.tile_pool(name="xp", bufs=2) as xp,
            tc.tile_pool(name="state", bufs=2) as statep,
            tc.tile_pool(name="work", bufs=2) as workp,
            tc.tile_pool(name="yb", bufs=2) as ybp,
            tc.tile_pool(name="ps_r", bufs=2, space="PSUM") as ps_r,
            tc.tile_pool(name="ps_z", bufs=2, space="PSUM") as ps_z,
            tc.tile_pool(name="ps_ghn", bufs=2, space="PSUM") as ps_ghn,
            tc.tile_pool(name="ps_gin", bufs=1, space="PSUM") as ps_gin,
            tc.tile_pool(name="ps_p", bufs=1, space="PSUM") as ps_p,
        ):
            whh_e = constp.tile([128, 1536], MM_DT)
            nc.sync.dma_start(whh_e[:], whh_e_d[:])
            whh_d = constp.tile([128, 1536], MM_DT)
            nc.sync.dma_start(whh_d[:], whh_d_d[:])
            whh_f = constp.tile([128, 1536], MM_DT)
            nc.sync.dma_start(whh_f[:], whh_f_d[:])
            wip_n = constp.tile([128, 512], MM_DT)
            nc.sync.dma_start(wip_n[:], wip_n_d[:])
            wih_e = constp.tile([CA, 768], MM_DT)
            nc.sync.dma_start(wih_e[:], wih_e_d[:])
            wih_d = constp.tile([CA, 768], MM_DT)
            nc.sync.dma_start(wih_d[:], wih_d_d[:])
            bhn_e = constp.tile([1, 256], MM_DT)
            nc.sync.dma_start(bhn_e[:], bhn_e_d[:])
            bhn_d = constp.tile([1, 256], MM_DT)
            nc.sync.dma_start(bhn_d[:], bhn_d_d[:])
            brz_f = constp.tile([1, 512], MM_DT)
            nc.sync.dma_start(brz_f[:], brz_f_d[:])
            bgin_f = constp.tile([1, 256], MM_DT)
            nc.sync.dma_start(bgin_f[:], bgin_f_d[:])
            projT = constp.tile([128, 64], MM_DT)
            nc.sync.dma_start(projT[:], projT_d[:])
            projb = constp.tile([32, 1], F32)
            nc.sync.dma_start(projb[:], projb_d[:])
            ones_row = constp.tile([1, BC], MM_DT)
            nc.vector.memset(ones_row[:], 1.0)
            dec_in0 = constp.tile([CA, BC], MM_DT)
            nc.vector.memset(dec_in0[0:C, :], 0.0)
            nc.vector.memset(dec_in0[C : C + 1, :], 1.0)

            def gru_v2(wh, h_prev, wi=None, x_ap=None, bhn=None,
                       wgin=None, brz=None, bgin=None, h_streams=None,
                       want_streams=False):
                """One GRU step. Encoder mode: wi/x_ap/bhn set (gi from x,
                biases ride x ones-row except bhn). Fused-decoder mode:
                wgin/brz/bgin/bhn set (everything from h_prev).
                h_streams=(c1,u): feed W@(c1+u) as two accumulated streams so
                the recurrence consumes u directly (h materialized off-chain).
                want_streams: return (h, c1, u) for the next step's streams."""
                p_r = ps_r.tile([128, 256], F32, name="p_r")
                p_z = ps_z.tile([128, 256], F32, name="p_z")
                p_ghn = ps_ghn.tile([128, 256], F32, name="p_ghn")
                p_gin = ps_gin.tile([128, 256], F32, name="p_gin")
                rhs_list = (
                    [h_streams[0][:, 0:128], h_streams[0][:, 128:256],
                     h_streams[1][:, 0:128], h_streams[1][:, 128:256]]
                    if h_streams is not None
                    else [h_prev[:, 0:128], h_prev[:, 128:256]]
                )

                def h_mms(m):
                    w0 = wh[:, (m * 2) * 128 : (m * 2 + 1) * 128]
                    w1 = wh[:, (m * 2 + 1) * 128 : (m * 2 + 2) * 128]
                    ws = [w0, w1] * (len(rhs_list) // 2)
                    return list(zip(ws, rhs_list))

                def rz_group(dst, m, bias_src):
                    seg = dst[:, (m % 2) * 128 : (m % 2 + 1) * 128]
                    ops = []
                    if bias_src is not None:
                        ops.append((bias_src[:, m * 128 : (m + 1) * 128],
                                    ones_row[:]))
                    if wi is not None:
                        ops.append((wi[:, m * 128 : (m + 1) * 128], x_ap))
                    ops.extend(h_mms(m))
                    for i, (lhsT, rhs) in enumerate(ops):
                        nc.tensor.matmul(seg, lhsT, rhs, start=(i == 0),
                                         stop=(i == len(ops) - 1))

                def ghn_group(cc):
                    seg = p_ghn[:, cc * 128 : (cc + 1) * 128]
                    ops = [(bhn[:, cc * 128 : (cc + 1) * 128], ones_row[:])]
                    ops.extend(h_mms(4 + cc))
                    for i, (lhsT, rhs) in enumerate(ops):
                        nc.tensor.matmul(seg, lhsT, rhs, start=(i == 0),
                                         stop=(i == len(ops) - 1))

                def gin_group(cc):
                    seg = p_gin[:, cc * 128 : (cc + 1) * 128]
                    if wgin is not None:
                        nc.tensor.matmul(seg, bgin[:, cc * 128 : (cc + 1) * 128],
                                         ones_row[:], start=True, stop=False)
                        nc.tensor.matmul(seg, wgin[:, (cc * 2) * 128 : (cc * 2 + 1) * 128],
                                         h_prev[:, 0:128], start=False, stop=False)
                        nc.tensor.matmul(seg, wgin[:, (cc * 2 + 1) * 128 : (cc * 2 + 2) * 128],
                                         h_prev[:, 128:256], start=False, stop=True)
                    else:
                        nc.tensor.matmul(seg, wi[:, (4 + cc) * 128 : (5 + cc) * 128],
                                         x_ap, start=True, stop=True)

                # PE order: r first (unblocks sig_r), then ghn (t1), then gin
                # (q), then z (sig_z is off-chain until the tanh window).
                rz_group(p_r, 0, brz)
                rz_group(p_r, 1, brz)
                ghn_group(0)
                ghn_group(1)
                gin_group(0)
                gin_group(1)
                rz_group(p_z, 2, brz)
                rz_group(p_z, 3, brz)

                r_t = workp.tile([128, 256], GATE_DT, name="r_t")
                nc.scalar.activation(r_t[:], p_r[:], AF.Sigmoid)
                t1 = workp.tile([128, 256], GATE_DT, name="t1")
                nc.vector.tensor_tensor(t1[:], p_ghn[:], r_t[:], OP.mult)
                z_t = workp.tile([128, 256], GATE_DT, name="z_t")
                nc.scalar.activation(z_t[:], p_z[:], AF.Sigmoid)
                q = workp.tile([128, 256], GATE_DT, name="q")
                nc.vector.tensor_tensor(q[:], t1[:], p_gin[:], OP.add)
                zb = workp.tile([128, 256], GATE_DT, name="zb")
                nc.vector.tensor_scalar(zb[:], z_t[:], -1.0, 1.0, OP.mult, OP.add)
                c1 = workp.tile([128, 256], GATE_DT, name="c1")
                nc.vector.tensor_tensor(c1[:], z_t[:], h_prev[:], OP.mult)
                n_t = workp.tile([128, 256], GATE_DT, name="n_t")
                nc.scalar.activation(n_t[:], q[:], AF.Tanh)
                u_t = workp.tile([128, 256], GATE_DT, name="u_t")
                nc.vector.tensor_tensor(u_t[:], zb[:], n_t[:], OP.mult)
                h_new = statep.tile([128, 256], GATE_DT, name="h")
                nc.vector.tensor_add(h_new[:], c1[:], u_t[:])
                if want_streams:
                    return h_new, c1, u_t
                return h_new

            def body():
                h = statep.tile([128, 256], GATE_DT, name="h")
                nc.vector.memset(h[:], 0.0)

                # ---- encoder ----
                streams = None
                for blk in range(nblk):
                    xb = xp.tile([CA, xblk * BC], MM_DT, name="xb")
                    nc.sync.dma_start(xb[:], xd[blk])
                    for j in range(xblk):
                        t = blk * xblk + j
                        if t >= t_steps:
                            break
                        x_ap = xb[:, j * BC : (j + 1) * BC]
                        if ENC_STREAM and t < t_steps - 1:
                            h, c1s, us = gru_v2(
                                whh_e, h, wi=wih_e, x_ap=x_ap, bhn=bhn_e,
                                h_streams=streams, want_streams=True)
                            streams = (c1s, us)
                        else:
                            h = gru_v2(whh_e, h, wi=wih_e, x_ap=x_ap,
                                       bhn=bhn_e, h_streams=streams)
                            streams = None

                # ---- decoder ----
                # proj/pred for step t is emitted AFTER step t+1's gru ops so
                # the proj matmuls never sit between h(t) and the next step's
                # r-matmuls in the PE FIFO.
                yblks = {}

                def emit_pred(t, h_t):
                    if t % YB == 0:
                        yblks[t // YB] = ybp.tile([32, YB * BC], F32, name="yblk")
                    yblk = yblks[t // YB]
                    psum_p = ps_p.tile([32, BC], F32, name="psum_p")
                    nc.tensor.matmul(psum_p[:], projT[:, 0:32], h_t[:, 0:128],
                                     start=True, stop=False)
                    nc.tensor.matmul(psum_p[:], projT[:, 32:64], h_t[:, 128:256],
                                     start=False, stop=True)
                    nc.scalar.activation(
                        yblk[:, (t % YB) * BC : (t % YB + 1) * BC], psum_p[:],
                        AF.Identity, bias=projb[:])
                    if t % YB == YB - 1:
                        nc.sync.dma_start(yd[t // YB], yblk[:])

                prev = None
                for t in range(t_steps):
                    if t == 0:
                        h = gru_v2(whh_d, h, wi=wih_d, x_ap=dec_in0[:],
                                   bhn=bhn_d)
                    else:
                        h = gru_v2(whh_f, h, wgin=wip_n, brz=brz_f,
                                   bgin=bgin_f, bhn=bhn_d)
                    if prev is not None:
                        emit_pred(t - 1, prev)
                    prev = h
                emit_pred(t_steps - 1, prev)

            if reps == 1:
                body()
            else:
                with tc.For_i(0, reps):
                    body()

    nc.compile()
    return nc


def prep_inputs_v2(x, enc_Wih, enc_Whh, enc_bih, enc_bhh,
                   dec_Wih, dec_Whh, dec_bih, dec_bhh, proj_W, proj_b,
                   t_steps=T):
    """Host-side shard + relayout for build_v2."""
    YB = 8
    nblk = (t_steps + XBLK - 1) // XBLK
    xblk = min(XBLK, t_steps)

    def whh_pack_f64(WT):
        # WT: [H, 3H] float64 (already transposed W.T)
        out = np.empty((128, 12 * 128), np.float64)
        for m in range(6):
            for k in range(2):
                out[:, (m * 2 + k) * 128 : (m * 2 + k + 1) * 128] = WT[
                    k * 128 : (k + 1) * 128, m * 128 : (m + 1) * 128
                ]
        return out

    def whh_pack(W):
        return whh_pack_f64(np.ascontiguousarray(W.T.astype(np.float64))).astype(NP_MM)

    def wih_pack(Wih, bih, bhh):
        fold = bih.astype(np.float64)
        fold[: 2 * H] += bhh[: 2 * H]
        Wa = np.concatenate([Wih.astype(np.float64), fold[:, None]], axis=1)
        return np.ascontiguousarray(Wa.T).astype(NP_MM)  # [CA, 3H]

    def proj_pack(W):
        out = np.empty((128, 64), np.float64)
        for k in range(2):
            out[:, 32 * k : 32 * k + 32] = W[:, k * 128 : (k + 1) * 128].T
        return out.astype(NP_MM)

    # ---- decoder fusion: gi_t = Wih @ (P h + pb) + bih  (t >= 2) ----
    WihP = dec_Wih.astype(np.float64) @ proj_W.astype(np.float64)  # [3H, H]
    bfold = dec_Wih.astype(np.float64) @ proj_b.astype(np.float64) + dec_bih.astype(np.float64)
    Wf = dec_Whh.astype(np.float64).copy()
    Wf[: 2 * H] += WihP[: 2 * H]  # rz fused; n-part of Whh stays pure
    whh_f = whh_pack_f64(np.ascontiguousarray(Wf.T)).astype(NP_MM)
    # (WihP)_n packed: chunk layout [cc*2 + k] -> WihP_n.T[k*128:(k+1)*128, cc*128:...]
    WipnT = np.ascontiguousarray(WihP[2 * H :].T)  # [H, H] = [256, 256]
    wip_n = np.empty((128, 4 * 128), np.float64)
    for cc in range(2):
        for k in range(2):
            wip_n[:, (cc * 2 + k) * 128 : (cc * 2 + k + 1) * 128] = WipnT[
                k * 128 : (k + 1) * 128, cc * 128 : (cc + 1) * 128
            ]
    brz_f = (bfold[: 2 * H] + dec_bhh.astype(np.float64)[: 2 * H])[None, :]
    bgin_f = bfold[2 * H :][None, :]

    shared = {
        "whh_e": whh_pack(enc_Whh),
        "whh_d": whh_pack(dec_Whh),
        "whh_f": whh_f,
        "wip_n": wip_n.astype(NP_MM),
        "wih_e": wih_pack(enc_Wih, enc_bih, enc_bhh),
        "wih_d": wih_pack(dec_Wih, dec_bih, dec_bhh),
        "bhn_e": np.ascontiguousarray(enc_bhh[2 * H :][None, :]).astype(NP_MM),
        "bhn_d": np.ascontiguousarray(dec_bhh[2 * H :][None, :]).astype(NP_MM),
        "brz_f": np.ascontiguousarray(brz_f).astype(NP_MM),
        "bgin_f": np.ascontiguousarray(bgin_f).astype(NP_MM),
        "projT": proj_pack(proj_W),
        "projb": np.ascontiguousarray(proj_b[:, None].astype(np.float32)),
    }
    in_maps = []
    for i in range(NCORES):
        xc = x[i * BC : (i + 1) * BC, :t_steps]  # [BC, t, C]
        xt = np.concatenate(
            [xc.transpose(1, 2, 0), np.ones((t_steps, 1, BC), np.float32)], axis=1
        )  # [t, CA, BC]
        xt = (
            xt.reshape(nblk, xblk, CA, BC)
            .transpose(0, 2, 1, 3)
            .reshape(nblk, CA, xblk * BC)
        )
        m = dict(shared)
        m["x_t"] = np.ascontiguousarray(xt).astype(NP_MM)
        in_maps.append(m)
    return in_maps


def unshard_v2(res, t_steps=T):
    YB = 8
    outs = []
    for i in range(NCORES):
        yt = res.results[i]["y_t"]  # [nyb, C, YB*BC]
        nyb = yt.shape[0]
        y = yt.reshape(nyb, C, YB, BC).transpose(0, 2, 3, 1).reshape(
            nyb * YB, BC, C
        )[:t_steps]  # [t, BC, C]
        outs.append(np.ascontiguousarray(y.transpose(1, 0, 2)))
    return np.concatenate(outs, axis=0)


def prep_inputs(x, enc_Wih, enc_Whh, enc_bih, enc_bhh,
                dec_Wih, dec_Whh, dec_bih, dec_bhh, proj_W, proj_b,
                t_steps=T):
    """Host-side shard + relayout. Returns in_maps (list of 8 dicts)."""
    nblk = (t_steps + XBLK - 1) // XBLK
    xblk = min(XBLK, t_steps)

    def whh_pack(W):
        WT = np.ascontiguousarray(W.T)  # [H, 3H]
        out = np.empty((128, 12 * 128), np.float32)
        for m in range(6):
            for k in range(2):
                out[:, (m * 2 + k) * 128 : (m * 2 + k + 1) * 128] = WT[
                    k * 128 : (k + 1) * 128, m * 128 : (m + 1) * 128
                ]
        return out.astype(NP_MM)

    def wih_pack(Wih, bih, bhh):
        fold = bih.astype(np.float64)
        fold[: 2 * H] += bhh[: 2 * H]
        Wa = np.concatenate([Wih.astype(np.float64), fold[:, None]], axis=1)
        return np.ascontiguousarray(Wa.T).astype(NP_MM)  # [CA, 3H]

    def proj_pack(W):
        out = np.empty((128, 64), np.float32)
        for k in range(2):
            out[:, 32 * k : 32 * k + 32] = W[:, k * 128 : (k + 1) * 128].T
        return out.astype(NP_MM)

    shared = {
        "whh_e": whh_pack(enc_Whh),
        "whh_d": whh_pack(dec_Whh),
        "wih_e": wih_pack(enc_Wih, enc_bih, enc_bhh),
        "wih_d": wih_pack(dec_Wih, dec_bih, dec_bhh),
        "bhn_e": np.ascontiguousarray(enc_bhh[2 * H :][None, :]).astype(NP_MM),
        "bhn_d": np.ascontiguousarray(dec_bhh[2 * H :][None, :]).astype(NP_MM),
        "projT": proj_pack(proj_W),
        "projb": np.ascontiguousarray(proj_b[:, None].astype(np.float32)),
    }
    in_maps = []
    for i in range(NCORES):
        xc = x[i * BC : (i + 1) * BC, :t_steps]  # [BC, t, C]
        xt = np.concatenate(
            [xc.transpose(1, 2, 0), np.ones((t_steps, 1, BC), np.float32)], axis=1
        )  # [t, CA, BC]
        xt = (
            xt.reshape(nblk, xblk, CA, BC)
            .transpose(0, 2, 1, 3)
            .reshape(nblk, CA, xblk * BC)
        )
        m = dict(shared)
        m["x_t"] = np.ascontiguousarray(xt).astype(NP_MM)
        in_maps.append(m)
    return in_maps


_BUILT = {}


BUILDER = build_v2
PREP = prep_inputs_v2


def run(inputs, t_steps=T, trace=False):
    if t_steps not in _BUILT:
        _BUILT[t_steps] = BUILDER(t_steps)
    nc = _BUILT[t_steps]
    in_maps = PREP(**inputs, t_steps=t_steps)
    res = run_bass_kernel_spmd(nc, in_maps, core_ids=list(range(NCORES)), trace=trace)
    if BUILDER is build_v2:
        return unshard_v2(res, t_steps=t_steps), res
    outs = []
    for i in range(NCORES):
        yt = res.results[i]["y_t"]  # [t, C, BC]
        outs.append(np.ascontiguousarray(yt.transpose(2, 0, 1)))  # [BC, t, C]
    y = np.concatenate(outs, axis=0)
    return y, res


def kernel(**inputs):
    y, _ = run(inputs, t_steps=T, trace=False)
    return y

